# revision 7
# baseline (speedup 1.0000x reference)
"""Llama4 MoE experts + shared LoRA, expert-parallel on 8 TRN2 NeuronCores.

Per-core (expert e): x[1024,1024] @ W_gu[1024,4096] (+ rank-8 LoRA) -> SwiGLU
-> h[1024,2048] @ W_dn[2048,1024] (+ rank-8 LoRA) -> out[1024,1024].

All matmul operands are bf16. Structure (v4):
- ALL weights are fetched with gpsimd SWDGE *casting* DMAs (f32 DRAM ->
  bf16 SBUF in one hop).  W_gu streams in quad chunks [128, 8, 512] so the
  destination runs are 1KB (256B runs measured ~70GB/s aggregate; packet
  overhead dominates below ~1KB).  W_dn tiles [128, 1024] give 2KB runs.
  No f32 weight staging and no cast instructions anywhere on the W path.
- The two HWDGE rings (sync, scalar) carry ONLY the x chain + outputs, so
  x -> bf16 -> XBAR-transpose completes as fast as the rings allow: loads
  split even/odd across the rings, casts on vector, transposed store on
  the opposite ring, 8 independent buffers.
- The tiny LoRA A/B matrices arrive host-marshalled in PE-ready layouts,
  combined into two casting DMAs; no PE transposes, no identity.
- Matmuls sharing a stationary operand issue back-to-back; PSUM f32;
  output stores spread across phase D on the sync ring.
"""
import sys

sys.path.insert(0, "/opt/trn_rl_repo")

import numpy as np

import concourse.bacc as bacc
import concourse.bass as bass
import concourse.mybir as mybir
import concourse.tile as tile
from concourse.bass_utils import run_bass_kernel_spmd

E = 8           # experts == cores
T = 1024        # tokens per expert
H = 1024        # hidden
I = 2048        # intermediate
F2 = 2 * I      # gate+up
R = 8           # lora rank
SCALING = 2.0   # lora_alpha / rank
P = 128         # partitions
NFREE = 512     # moving free-dim per matmul (one PSUM bank of fp32)
KH = H // P     # 8 k-tiles over H
KI = I // P     # 16 k-tiles over I
NT = T // NFREE     # 2 T-chunks
NH = H // NFREE     # 2 H-chunks
NFP = I // P        # 16 F-pair tiles (gate i pairs with up i+16)
QW = 512            # W_gu quad width (4 f-tiles)
NQ = I // QW        # 4 quads each for gate / up halves -> 8 total... (I/QW=4)

F32 = mybir.dt.float32
BF16 = mybir.dt.bfloat16


def build_kernel():
    nc = bacc.Bacc("TRN2", target_bir_lowering=False, debug=False)

    x_d = nc.dram_tensor("x", [T, H], F32, kind="ExternalInput")
    wgu_d = nc.dram_tensor("w_gu", [H, F2], F32, kind="ExternalInput")
    wdn_d = nc.dram_tensor("w_dn", [I, H], F32, kind="ExternalInput")
    # lora mats arrive combined + in PE-ready layouts (host marshalling):
    #   smA[p, 8k+r]        = A_gu[r, 128k+p] for cols 0:64, A_dn for 64:192
    #   smB[r, 0:4096]      = B_gu.T ; smB[r, 4096:5120] = B_dn.T
    smA_d = nc.dram_tensor("smA", [P, R * (KH + KI)], F32, kind="ExternalInput")
    smB_d = nc.dram_tensor("smB", [R, F2 + H], F32, kind="ExternalInput")
    out_d = nc.dram_tensor("out", [T, H], F32, kind="ExternalOutput")

    with tile.TileContext(nc) as tc:
        with (
            tc.tile_pool(name="xT", bufs=1) as xT_pool,
            tc.tile_pool(name="hT", bufs=1) as hT_pool,
            tc.tile_pool(name="wdnb", bufs=1) as wdnb_pool,
            tc.tile_pool(name="smalls", bufs=1) as small_pool,
            tc.tile_pool(name="xnat", bufs=8) as xnat_pool,
            tc.tile_pool(name="xbf", bufs=8) as xbf_pool,
            tc.tile_pool(name="wq", bufs=4) as wq_pool,
            tc.tile_pool(name="silu", bufs=2) as silu_pool,
            tc.tile_pool(name="outs", bufs=4) as out_pool,
            tc.tile_pool(name="ps_mm", bufs=6, space="PSUM") as ps_mm,
            tc.tile_pool(name="ps_r2", bufs=2, space="PSUM") as ps_r2,
        ):
            # ---- combined lora mats: 2 casting DMAs on the gpsimd ring ----
            smAb = small_pool.tile([P, R * (KH + KI)], BF16, tag="smAb")
            smBb = small_pool.tile([R, F2 + H], BF16, tag="smBb")
            nc.gpsimd.dma_start(smAb[:], smA_d[:])
            nc.gpsimd.dma_start(smBb[:], smB_d[:])
            aguLb = smAb[:, 0:R * KH]
            adnLb = smAb[:, R * KH:R * (KH + KI)]
            bguTb = smBb[:, 0:F2]
            bdnTb = smBb[:, F2:F2 + H]

            # ---- W_gu quads: gpsimd casting DMA, [128, 8, 512] bf16 ----
            wq_tiles = {}

            def emit_quad(q):
                fg, fu = QW * q, QW * q + I
                wqg = wq_pool.tile([P, KH, QW], BF16, tag="wq", name=f"wqg{q}")
                wqu = wq_pool.tile([P, KH, QW], BF16, tag="wq", name=f"wqu{q}")
                nc.gpsimd.dma_start(
                    wqg[:], wgu_d[:, fg:fg + QW].rearrange("(ko ki) f -> ki ko f", ki=P))
                nc.gpsimd.dma_start(
                    wqu[:], wgu_d[:, fu:fu + QW].rearrange("(ko ki) f -> ki ko f", ki=P))
                wq_tiles[q] = (wqg, wqu)

            # ---- x chain split across both HWDGE rings ----
            xT = xT_pool.tile([P, KH, T], BF16, tag="xT", name="xT")
            xbf_tiles = {}

            def x_load(b):
                eng = nc.sync if b % 2 == 0 else nc.scalar
                xb = xnat_pool.tile([P, H], F32)
                eng.dma_start(xb[:], x_d[P * b:P * (b + 1), :])
                xbf = xbf_pool.tile([P, H], BF16)
                nc.vector.tensor_copy(xbf[:], xb[:])
                xbf_tiles[b] = xbf

            def x_store(b):
                eng = nc.scalar if b % 2 == 0 else nc.sync
                eng.dma_start(xT[:, :, P * b:P * (b + 1)], xbf_tiles.pop(b)[:],
                              transpose=True)

            emit_quad(0)
            x_load(0); x_load(1); x_load(2); x_load(3)
            x_store(0); x_store(1); x_store(2); x_store(3)
            x_load(4); x_load(5); x_load(6); x_load(7)
            x_store(4); x_store(5); x_store(6); x_store(7)
            emit_quad(1)

            # ---- r1T = SCALING * (A_gu @ x^T): [8 R, 1024 T] bf16 ----
            r1T = small_pool.tile([R, T], BF16, tag="r1T")

            def emit_r1(t):
                rps = ps_mm.tile([R, NFREE], F32, tag="mm", name=f"r1ps{t}")
                for k in range(KH):
                    nc.tensor.matmul(rps[:], aguLb[:, R * k:R * (k + 1)],
                                     xT[:, k, NFREE * t:NFREE * (t + 1)],
                                     start=(k == 0), stop=(k == KH - 1))
                nc.vector.tensor_scalar_mul(
                    r1T[:, NFREE * t:NFREE * (t + 1)], rps[:], SCALING)

            # ---- phase B: gate_up^T + SwiGLU -> hT[k] [128 I, 1024 T] ----
            hT = [hT_pool.tile([P, T], BF16, tag=f"hT{k}", name=f"hT{k}") for k in range(KI)]
            wdnb = [wdnb_pool.tile([P, H], BF16, tag=f"wdnb{k}", name=f"wdnb{k}")
                    for k in range(KI)]

            def quad_tail(q):
                # prefetch the quad pair two ahead; emitted AFTER quad q's
                # blocks so the buffer-reuse WAR edge (new DMA vs q's
                # matmuls) is anchored on already-emitted readers.  W_dn
                # rows (bf16 via casting DMA) ride the same gpsimd ring.
                if q + 2 <= 3:
                    emit_quad(q + 2)
                if 1 <= q < 3:
                    for k in range(8 * (q - 1), 8 * q):
                        nc.gpsimd.dma_start(wdnb[k][:], wdn_d[P * k:P * (k + 1), :])

            def iter_block(i, t):
                q, fj = i // 4, i % 4
                wqg, wqu = wq_tiles[q]
                fs = slice(P * fj, P * (fj + 1))
                fg, fu = P * i, P * i + I
                ts = slice(NFREE * t, NFREE * (t + 1))
                psg = ps_mm.tile([P, NFREE], F32, tag="mm", name=f"psg{i}_{t}")
                psu = ps_mm.tile([P, NFREE], F32, tag="mm", name=f"psu{i}_{t}")
                for k in range(KH):
                    nc.tensor.matmul(psg[:], wqg[:, k, fs],
                                     xT[:, k, ts], start=(k == 0), stop=False)
                nc.tensor.matmul(psg[:], bguTb[:, fg:fg + P], r1T[:, ts],
                                 start=False, stop=True)
                for k in range(KH):
                    nc.tensor.matmul(psu[:], wqu[:, k, fs],
                                     xT[:, k, ts], start=(k == 0), stop=False)
                nc.tensor.matmul(psu[:], bguTb[:, fu:fu + P], r1T[:, ts],
                                 start=False, stop=True)
                sg = silu_pool.tile([P, NFREE], F32, tag="silu")
                nc.scalar.activation(sg[:], psg[:],
                                     mybir.ActivationFunctionType.Silu)
                nc.vector.tensor_mul(hT[i][:, ts], sg[:], psu[:])

            # q=0 runs t=0 for i=0,1 first so the PE starts on the earliest
            # x half; r1(1) slots in before the t=1 blocks.
            emit_r1(0)
            iter_block(0, 0)
            iter_block(1, 0)
            emit_r1(1)
            iter_block(0, 1)
            iter_block(1, 1)
            iter_block(2, 0)
            iter_block(2, 1)
            iter_block(3, 0)
            iter_block(3, 1)
            quad_tail(0)
            for q in range(1, 4):
                for fj in range(4):
                    i = 4 * q + fj
                    iter_block(i, 0)
                    iter_block(i, 1)
                quad_tail(q)

            # ---- r2T = SCALING * (A_dn @ hidden^T): [8 R, 1024 T] bf16 ----
            r2T = small_pool.tile([R, T], BF16, tag="r2T")
            rps2 = [ps_r2.tile([R, NFREE], F32, tag="r2", name=f"r2ps{t}")
                    for t in range(NT)]
            for k in range(KI):
                for t in range(NT):
                    nc.tensor.matmul(rps2[t][:], adnLb[:, R * k:R * (k + 1)],
                                     hT[k][:, NFREE * t:NFREE * (t + 1)],
                                     start=(k == 0), stop=(k == KI - 1))
            for t in range(NT):
                nc.vector.tensor_scalar_mul(
                    r2T[:, NFREE * t:NFREE * (t + 1)], rps2[t][:], SCALING)

            # ---- phase D: out[T, H] = hidden @ W_dn + lora ----
            # wdnb[k] resident bf16: pure matmul stream, no DMA/cast here.
            nout = 0
            for grp in range(4):
                pos = [[ps_mm.tile([P, NFREE], F32, tag="mm",
                                   name=f"po{grp}_{jj}_{h}")
                        for h in range(NH)] for jj in range(2)]
                for k in range(KI):
                    for jj in range(2):
                        j = 2 * grp + jj
                        for h in range(NH):
                            hs = slice(NFREE * h, NFREE * (h + 1))
                            nc.tensor.matmul(pos[jj][h][:],
                                             hT[k][:, P * j:P * (j + 1)],
                                             wdnb[k][:, hs],
                                             start=(k == 0), stop=False)
                for jj in range(2):
                    j = 2 * grp + jj
                    for h in range(NH):
                        hs = slice(NFREE * h, NFREE * (h + 1))
                        nc.tensor.matmul(pos[jj][h][:], r2T[:, P * j:P * (j + 1)],
                                         bdnTb[:, hs], start=False, stop=True)
                        ot = out_pool.tile([P, NFREE], F32, tag="outs")
                        if nout % 2 == 0:
                            nc.scalar.activation(ot[:], pos[jj][h][:],
                                                 mybir.ActivationFunctionType.Copy)
                        else:
                            nc.vector.tensor_copy(ot[:], pos[jj][h][:])
                        nout += 1
                        nc.sync.dma_start(out_d[P * j:P * (j + 1), hs], ot[:])

    nc.finalize()
    return nc


_NC_CACHE = None


def _get_nc():
    global _NC_CACHE
    if _NC_CACHE is None:
        _NC_CACHE = build_kernel()
    return _NC_CACHE


def _run(hidden_states, gate_up_proj, down_proj,
         lora_A_gu, lora_B_gu, lora_A_dn, lora_B_dn, **spmd_kwargs):
    f32 = np.float32
    hidden_states = np.ascontiguousarray(hidden_states, dtype=f32)
    gate_up_proj = np.ascontiguousarray(gate_up_proj, dtype=f32)
    down_proj = np.ascontiguousarray(down_proj, dtype=f32)
    lora_A_gu = np.asarray(lora_A_gu, dtype=f32)
    lora_B_gu = np.asarray(lora_B_gu, dtype=f32)
    lora_A_dn = np.asarray(lora_A_dn, dtype=f32)
    lora_B_dn = np.asarray(lora_B_dn, dtype=f32)

    # host-side layout marshalling of the tiny (<=128KB) lora mats:
    # smA[p, R*k+r] = A[r, 128k+p] (gu cols then dn cols); smB = [B_gu.T | B_dn.T]
    a_guL = lora_A_gu.reshape(R, KH, P).transpose(2, 1, 0).reshape(P, R * KH)
    a_dnL = lora_A_dn.reshape(R, KI, P).transpose(2, 1, 0).reshape(P, R * KI)
    smA = np.ascontiguousarray(np.concatenate([a_guL, a_dnL], axis=1))
    smB = np.ascontiguousarray(np.concatenate([lora_B_gu.T, lora_B_dn.T], axis=1))

    nc = _get_nc()
    in_maps = []
    for e in range(E):
        in_maps.append({
            "x": hidden_states[T * e:T * (e + 1), :],
            "w_gu": gate_up_proj[e],
            "w_dn": down_proj[e],
            "smA": smA,
            "smB": smB,
        })
    res = run_bass_kernel_spmd(nc, in_maps, core_ids=list(range(E)),
                               **spmd_kwargs)
    out = np.concatenate([res.results[e]["out"] for e in range(E)], axis=0)
    return out, res


def kernel(hidden_states, gate_up_proj, down_proj,
           lora_A_gu, lora_B_gu, lora_A_dn, lora_B_dn):
    out, _ = _run(hidden_states, gate_up_proj, down_proj,
                  lora_A_gu, lora_B_gu, lora_A_dn, lora_B_dn)
    return out


# revision 8
# speedup vs baseline: 1.0743x; 1.0743x over previous
"""Llama4 MoE experts + shared LoRA, expert-parallel on 8 TRN2 NeuronCores.

Per-core (expert e): x[1024,1024] @ W_gu[1024,4096] (+ rank-8 LoRA) -> SwiGLU
-> h[1024,2048] @ W_dn[2048,1024] (+ rank-8 LoRA) -> out[1024,1024].

All matmul operands are bf16. Structure (v5):
- x -> bf16 -> XBAR-transpose chain is the first thing on BOTH HWDGE
  rings (loads even/odd split, casts on vector, transposed store on the
  opposite ring, 8 independent buffers) so xT is ready ASAP.
- W_gu pair 0 + all of W_dn + the lora mats ride the gpsimd SWDGE ring as
  *casting* DMAs (f32 DRAM -> bf16 SBUF directly), in parallel with the x
  chain.  W_gu pairs 1-15 stream on the sync ring as f32 (512B
  descriptors) + vector cast, paced two pairs ahead with the prefetch
  emitted AFTER the blocks whose buffers it reuses.
- The tiny LoRA A/B matrices arrive host-marshalled in PE-ready layouts
  (combined into two casting DMAs); no PE transposes, no identity.
- r2 (A_dn @ h^T) accumulates inside phase B as each hT tile is produced,
  so the B->D boundary has no serial r2 pass.
- Phase D runs 8 single-j-column groups so output stores spread evenly
  and the final-store tail is short.  PSUM f32 throughout.
"""
import sys

sys.path.insert(0, "/opt/trn_rl_repo")

import numpy as np

import concourse.bacc as bacc
import concourse.bass as bass
import concourse.mybir as mybir
import concourse.tile as tile
from concourse.bass_utils import run_bass_kernel_spmd

E = 8           # experts == cores
T = 1024        # tokens per expert
H = 1024        # hidden
I = 2048        # intermediate
F2 = 2 * I      # gate+up
R = 8           # lora rank
SCALING = 2.0   # lora_alpha / rank
P = 128         # partitions
NFREE = 512     # moving free-dim per matmul (one PSUM bank of fp32)
KH = H // P     # 8 k-tiles over H
KI = I // P     # 16 k-tiles over I
NT = T // NFREE     # 2 T-chunks
NH = H // NFREE     # 2 H-chunks
NFP = I // P        # 16 F-pair tiles (gate i pairs with up i+16)

F32 = mybir.dt.float32
BF16 = mybir.dt.bfloat16


def build_kernel():
    nc = bacc.Bacc("TRN2", target_bir_lowering=False, debug=False)

    x_d = nc.dram_tensor("x", [T, H], F32, kind="ExternalInput")
    wgu_d = nc.dram_tensor("w_gu", [H, F2], F32, kind="ExternalInput")
    wdn_d = nc.dram_tensor("w_dn", [I, H], F32, kind="ExternalInput")
    # lora mats arrive combined + in PE-ready layouts (host marshalling):
    #   smA[p, 8k+r]   = A_gu[r, 128k+p] for cols 0:64, A_dn for 64:192
    #   smB[r, 0:4096] = B_gu.T ; smB[r, 4096:5120] = B_dn.T
    smA_d = nc.dram_tensor("smA", [P, R * (KH + KI)], F32, kind="ExternalInput")
    smB_d = nc.dram_tensor("smB", [R, F2 + H], F32, kind="ExternalInput")
    out_d = nc.dram_tensor("out", [T, H], F32, kind="ExternalOutput")

    with tile.TileContext(nc) as tc:
        with (
            tc.tile_pool(name="xT", bufs=1) as xT_pool,
            tc.tile_pool(name="hT", bufs=1) as hT_pool,
            tc.tile_pool(name="wdnb", bufs=1) as wdnb_pool,
            tc.tile_pool(name="smalls", bufs=1) as small_pool,
            tc.tile_pool(name="xnat", bufs=8) as xnat_pool,
            tc.tile_pool(name="xbf", bufs=8) as xbf_pool,
            tc.tile_pool(name="wgu", bufs=6) as wgu_pool,
            tc.tile_pool(name="wgub", bufs=6) as wgub_pool,
            tc.tile_pool(name="silu", bufs=2) as silu_pool,
            tc.tile_pool(name="outs", bufs=4) as out_pool,
            tc.tile_pool(name="ps_mm", bufs=6, space="PSUM") as ps_mm,
            tc.tile_pool(name="ps_r2", bufs=2, space="PSUM") as ps_r2,
        ):
            # ---- combined lora mats: 2 casting DMAs on the gpsimd ring ----
            smAb = small_pool.tile([P, R * (KH + KI)], BF16, tag="smAb")
            smBb = small_pool.tile([R, F2 + H], BF16, tag="smBb")
            nc.gpsimd.dma_start(smAb[:], smA_d[:])
            nc.gpsimd.dma_start(smBb[:], smB_d[:])
            aguLb = smAb[:, 0:R * KH]
            adnLb = smAb[:, R * KH:R * (KH + KI)]
            bguTb = smBb[:, 0:F2]
            bdnTb = smBb[:, F2:F2 + H]

            # ---- x chain first on both HWDGE rings ----
            xT = xT_pool.tile([P, KH, T], BF16, tag="xT", name="xT")
            xbf_tiles = {}

            def x_load(b):
                eng = nc.sync if b % 2 == 0 else nc.scalar
                xb = xnat_pool.tile([P, H], F32)
                eng.dma_start(xb[:], x_d[P * b:P * (b + 1), :])
                xbf = xbf_pool.tile([P, H], BF16)
                nc.vector.tensor_copy(xbf[:], xb[:])
                xbf_tiles[b] = xbf

            def x_store(b):
                eng = nc.scalar if b % 2 == 0 else nc.sync
                eng.dma_start(xT[:, :, P * b:P * (b + 1)], xbf_tiles.pop(b)[:],
                              transpose=True)

            x_load(0); x_load(1); x_load(2); x_load(3)
            x_store(0); x_store(1); x_store(2); x_store(3)
            x_load(4); x_load(5); x_load(6); x_load(7)
            x_store(4); x_store(5); x_store(6); x_store(7)

            # ---- W_gu pair 0: gpsimd casting DMA (parallel with x chain) --
            wgu_stage = {}
            wgub_tiles = {}

            def w_src(f0):
                return wgu_d[:, f0:f0 + P].rearrange("(ko ki) f -> ki ko f", ki=P)

            wgb0 = wgub_pool.tile([P, KH, P], BF16, tag="wgub", name="wgb0")
            wub0 = wgub_pool.tile([P, KH, P], BF16, tag="wgub", name="wub0")
            nc.gpsimd.dma_start(wgb0[:], w_src(0))
            nc.gpsimd.dma_start(wub0[:], w_src(I))
            wgub_tiles[0] = (wgb0, wub0)

            def emit_w_dma(i):
                wg = wgu_pool.tile([P, KH, P], F32, tag="wgu", name=f"wg{i}")
                wu = wgu_pool.tile([P, KH, P], F32, tag="wgu", name=f"wu{i}")
                nc.sync.dma_start(wg[:], w_src(P * i))
                nc.sync.dma_start(wu[:], w_src(P * i + I))
                wgu_stage[i] = (wg, wu)

            def emit_w_cast(i):
                wg, wu = wgu_stage.pop(i)
                wgb = wgub_pool.tile([P, KH, P], BF16, tag="wgub", name=f"wgb{i}")
                wub = wgub_pool.tile([P, KH, P], BF16, tag="wgub", name=f"wub{i}")
                nc.vector.tensor_copy(wgb[:], wg[:])
                nc.vector.tensor_copy(wub[:], wu[:])
                wgub_tiles[i] = (wgb, wub)

            emit_w_dma(1)
            emit_w_dma(2)

            # ---- r1T = SCALING * (A_gu @ x^T): [8 R, 1024 T] bf16 ----
            r1T = small_pool.tile([R, T], BF16, tag="r1T")

            def emit_r1(t):
                rps = ps_mm.tile([R, NFREE], F32, tag="mm", name=f"r1ps{t}")
                for k in range(KH):
                    nc.tensor.matmul(rps[:], aguLb[:, R * k:R * (k + 1)],
                                     xT[:, k, NFREE * t:NFREE * (t + 1)],
                                     start=(k == 0), stop=(k == KH - 1))
                nc.vector.tensor_scalar_mul(
                    r1T[:, NFREE * t:NFREE * (t + 1)], rps[:], SCALING)

            # ---- phase B: gate_up^T + SwiGLU -> hT[k]; r2 folded in ----
            hT = [hT_pool.tile([P, T], BF16, tag=f"hT{k}", name=f"hT{k}") for k in range(KI)]
            wdnb = [wdnb_pool.tile([P, H], BF16, tag=f"wdnb{k}", name=f"wdnb{k}")
                    for k in range(KI)]
            r2T = small_pool.tile([R, T], BF16, tag="r2T")
            rps2 = [ps_r2.tile([R, NFREE], F32, tag="r2", name=f"r2ps{t}")
                    for t in range(NT)]

            def iter_block(i, t):
                fg, fu = P * i, P * i + I
                wgb, wub = wgub_tiles[i]
                ts = slice(NFREE * t, NFREE * (t + 1))
                psg = ps_mm.tile([P, NFREE], F32, tag="mm", name=f"psg{i}_{t}")
                psu = ps_mm.tile([P, NFREE], F32, tag="mm", name=f"psu{i}_{t}")
                for k in range(KH):
                    nc.tensor.matmul(psg[:], wgb[:, k, :],
                                     xT[:, k, ts], start=(k == 0), stop=False)
                nc.tensor.matmul(psg[:], bguTb[:, fg:fg + P], r1T[:, ts],
                                 start=False, stop=True)
                for k in range(KH):
                    nc.tensor.matmul(psu[:], wub[:, k, :],
                                     xT[:, k, ts], start=(k == 0), stop=False)
                nc.tensor.matmul(psu[:], bguTb[:, fu:fu + P], r1T[:, ts],
                                 start=False, stop=True)
                sg = silu_pool.tile([P, NFREE], F32, tag="silu")
                nc.scalar.activation(sg[:], psg[:],
                                     mybir.ActivationFunctionType.Silu)
                nc.vector.tensor_mul(hT[i][:, ts], sg[:], psu[:])

            def iter_tail(i):
                # prefetch W pair i+2 (after the blocks whose wgu-staging
                # slots it reuses have been emitted), pace W_dn casting DMAs
                # on the gpsimd ring, and fold iter i's r2 accumulation in.
                if i + 2 <= 15:
                    emit_w_dma(i + 2)
                if 2 <= i < 10:
                    for k in (2 * (i - 2), 2 * (i - 2) + 1):
                        nc.gpsimd.dma_start(wdnb[k][:], wdn_d[P * k:P * (k + 1), :])
                for t in range(NT):
                    nc.tensor.matmul(rps2[t][:], adnLb[:, R * i:R * (i + 1)],
                                     hT[i][:, NFREE * t:NFREE * (t + 1)],
                                     start=(i == 0), stop=(i == KI - 1))

            emit_r1(0)
            iter_block(0, 0)
            emit_w_cast(1)
            iter_block(1, 0)
            emit_r1(1)
            iter_block(0, 1)
            iter_block(1, 1)
            iter_tail(0)
            iter_tail(1)
            for i in range(2, NFP):
                emit_w_cast(i)
                iter_block(i, 0)
                iter_block(i, 1)
                iter_tail(i)

            for t in range(NT):
                nc.vector.tensor_scalar_mul(
                    r2T[:, NFREE * t:NFREE * (t + 1)], rps2[t][:], SCALING)

            # ---- phase D: out[T, H] = hidden @ W_dn + lora ----
            # 8 groups of one T-column each: stores spread evenly, short tail.
            nout = 0
            for j in range(8):
                po = [ps_mm.tile([P, NFREE], F32, tag="mm", name=f"po{j}_{h}")
                      for h in range(NH)]
                for k in range(KI):
                    for h in range(NH):
                        hs = slice(NFREE * h, NFREE * (h + 1))
                        nc.tensor.matmul(po[h][:],
                                         hT[k][:, P * j:P * (j + 1)],
                                         wdnb[k][:, hs],
                                         start=(k == 0), stop=False)
                for h in range(NH):
                    hs = slice(NFREE * h, NFREE * (h + 1))
                    nc.tensor.matmul(po[h][:], r2T[:, P * j:P * (j + 1)],
                                     bdnTb[:, hs], start=False, stop=True)
                    ot = out_pool.tile([P, NFREE], F32, tag="outs")
                    if nout % 2 == 0:
                        nc.scalar.activation(ot[:], po[h][:],
                                             mybir.ActivationFunctionType.Copy)
                    else:
                        nc.vector.tensor_copy(ot[:], po[h][:])
                    nout += 1
                    nc.sync.dma_start(out_d[P * j:P * (j + 1), hs], ot[:])

    nc.finalize()
    return nc


_NC_CACHE = None


def _get_nc():
    global _NC_CACHE
    if _NC_CACHE is None:
        _NC_CACHE = build_kernel()
    return _NC_CACHE


def _run(hidden_states, gate_up_proj, down_proj,
         lora_A_gu, lora_B_gu, lora_A_dn, lora_B_dn, **spmd_kwargs):
    f32 = np.float32
    hidden_states = np.ascontiguousarray(hidden_states, dtype=f32)
    gate_up_proj = np.ascontiguousarray(gate_up_proj, dtype=f32)
    down_proj = np.ascontiguousarray(down_proj, dtype=f32)
    lora_A_gu = np.asarray(lora_A_gu, dtype=f32)
    lora_B_gu = np.asarray(lora_B_gu, dtype=f32)
    lora_A_dn = np.asarray(lora_A_dn, dtype=f32)
    lora_B_dn = np.asarray(lora_B_dn, dtype=f32)

    # host-side layout marshalling of the tiny (<=128KB) lora mats:
    # smA[p, R*k+r] = A[r, 128k+p] (gu cols then dn cols); smB = [B_gu.T | B_dn.T]
    a_guL = lora_A_gu.reshape(R, KH, P).transpose(2, 1, 0).reshape(P, R * KH)
    a_dnL = lora_A_dn.reshape(R, KI, P).transpose(2, 1, 0).reshape(P, R * KI)
    smA = np.ascontiguousarray(np.concatenate([a_guL, a_dnL], axis=1))
    smB = np.ascontiguousarray(np.concatenate([lora_B_gu.T, lora_B_dn.T], axis=1))

    nc = _get_nc()
    in_maps = []
    for e in range(E):
        in_maps.append({
            "x": hidden_states[T * e:T * (e + 1), :],
            "w_gu": gate_up_proj[e],
            "w_dn": down_proj[e],
            "smA": smA,
            "smB": smB,
        })
    res = run_bass_kernel_spmd(nc, in_maps, core_ids=list(range(E)),
                               **spmd_kwargs)
    out = np.concatenate([res.results[e]["out"] for e in range(E)], axis=0)
    return out, res


def kernel(hidden_states, gate_up_proj, down_proj,
           lora_A_gu, lora_B_gu, lora_A_dn, lora_B_dn):
    out, _ = _run(hidden_states, gate_up_proj, down_proj,
                  lora_A_gu, lora_B_gu, lora_A_dn, lora_B_dn)
    return out


# revision 16
# speedup vs baseline: 1.1257x; 1.0478x over previous
"""Llama4 MoE experts + shared LoRA, expert-parallel on 8 TRN2 NeuronCores.

Per-core (expert e): x[1024,1024] @ W_gu[1024,4096] (+ rank-8 LoRA) -> SwiGLU
-> h[1024,2048] @ W_dn[2048,1024] (+ rank-8 LoRA) -> out[1024,1024].

All matmul operands are bf16. Structure (v5):
- x -> bf16 -> XBAR-transpose chain is the first thing on BOTH HWDGE
  rings (loads even/odd split, casts on vector, transposed store on the
  opposite ring, 8 independent buffers) so xT is ready ASAP.
- W_gu pair 0 + all of W_dn + the lora mats ride the gpsimd SWDGE ring as
  *casting* DMAs (f32 DRAM -> bf16 SBUF directly), in parallel with the x
  chain.  W_gu pairs 1-15 stream on the sync ring as f32 (512B
  descriptors) + vector cast, paced two pairs ahead with the prefetch
  emitted AFTER the blocks whose buffers it reuses.
- The tiny LoRA A/B matrices arrive host-marshalled in PE-ready layouts
  (combined into two casting DMAs); no PE transposes, no identity.
- r2 (A_dn @ h^T) accumulates inside phase B as each hT tile is produced,
  so the B->D boundary has no serial r2 pass.
- Phase D runs 8 single-j-column groups so output stores spread evenly
  and the final-store tail is short.  PSUM f32 throughout.
"""
import sys

sys.path.insert(0, "/opt/trn_rl_repo")

import numpy as np

import concourse.bacc as bacc
import concourse.bass as bass
import concourse.mybir as mybir
import concourse.tile as tile
from concourse.bass_utils import run_bass_kernel_spmd
from concourse.masks import make_identity

E = 8           # experts == cores
T = 1024        # tokens per expert
H = 1024        # hidden
I = 2048        # intermediate
F2 = 2 * I      # gate+up
R = 8           # lora rank
SCALING = 2.0   # lora_alpha / rank
P = 128         # partitions
NFREE = 512     # moving free-dim per matmul (one PSUM bank of fp32)
KH = H // P     # 8 k-tiles over H
KI = I // P     # 16 k-tiles over I
NT = T // NFREE     # 2 T-chunks
NH = H // NFREE     # 2 H-chunks
NFP = I // P        # 16 F-pair tiles (gate i pairs with up i+16)

F32 = mybir.dt.float32
BF16 = mybir.dt.bfloat16


def build_kernel():
    nc = bacc.Bacc("TRN2", target_bir_lowering=False, debug=False)

    x_d = nc.dram_tensor("x", [T, H], F32, kind="ExternalInput")
    wgu_d = nc.dram_tensor("w_gu", [H, F2], F32, kind="ExternalInput")
    wdn_d = nc.dram_tensor("w_dn", [I, H], F32, kind="ExternalInput")
    # lora mats arrive combined + in PE-ready layouts (host marshalling):
    #   smA[p, 8k+r]   = A_gu[r, 128k+p] for cols 0:64, A_dn for 64:192
    #   smB[r, 0:4096] = B_gu.T ; smB[r, 4096:5120] = B_dn.T
    smA_d = nc.dram_tensor("smA", [P, R * (KH + KI)], F32, kind="ExternalInput")
    smB_d = nc.dram_tensor("smB", [R, F2 + H], F32, kind="ExternalInput")
    out_d = nc.dram_tensor("out", [T, H], F32, kind="ExternalOutput")

    with tile.TileContext(nc) as tc:
        with (
            tc.tile_pool(name="xT", bufs=1) as xT_pool,
            tc.tile_pool(name="hT", bufs=1) as hT_pool,
            tc.tile_pool(name="wdnb", bufs=1) as wdnb_pool,
            tc.tile_pool(name="smalls", bufs=1) as small_pool,
            tc.tile_pool(name="xnat", bufs=8) as xnat_pool,
            tc.tile_pool(name="xbf", bufs=8) as xbf_pool,
            tc.tile_pool(name="wgu", bufs=6) as wgu_pool,
            tc.tile_pool(name="wgub", bufs=8) as wgub_pool,
            tc.tile_pool(name="silu", bufs=2) as silu_pool,
            tc.tile_pool(name="outs", bufs=4) as out_pool,
            tc.tile_pool(name="ps_mm", bufs=6, space="PSUM") as ps_mm,
            tc.tile_pool(name="ps_r2", bufs=2, space="PSUM") as ps_r2,
        ):
            # ---- combined lora mats: 2 casting DMAs on the gpsimd ring ----
            smAb = small_pool.tile([P, R * (KH + KI)], BF16, tag="smAb")
            smBb = small_pool.tile([R, F2 + H], BF16, tag="smBb")
            nc.gpsimd.dma_start(smAb[:], smA_d[:])
            nc.gpsimd.dma_start(smBb[:], smB_d[:])
            aguLb = smAb[:, 0:R * KH]
            adnLb = smAb[:, R * KH:R * (KH + KI)]
            bguTb = smBb[:, 0:F2]
            bdnTb = smBb[:, F2:F2 + H]

            # ---- x chain first on both HWDGE rings ----
            xT = xT_pool.tile([P, KH, T], BF16, tag="xT", name="xT")
            xbf_tiles = {}

            def x_load(b):
                eng = nc.sync if b % 2 == 0 else nc.scalar
                xb = xnat_pool.tile([P, H], F32)
                eng.dma_start(xb[:], x_d[P * b:P * (b + 1), :])
                xbf = xbf_pool.tile([P, H], BF16)
                nc.vector.tensor_copy(xbf[:], xb[:])
                xbf_tiles[b] = xbf

            def x_store(b):
                # whole-block XBAR transpose (the only HW-proven form); each
                # instruction streams ~512B packets at ~30GB/s, so only the
                # t=1 half (blocks 4-7) goes through the XBAR — the t=0 half
                # is transposed on the PE for fast availability.
                eng = nc.scalar if b % 2 == 0 else nc.sync
                eng.dma_start(xT[:, :, P * b:P * (b + 1)], xbf_tiles.pop(b)[:],
                              transpose=True)

            def x_transpose_pe(b, ident):
                # 8 PE transposes [128,128] bf16 + copies spread over 3 engines
                xbf = xbf_tiles.pop(b)
                for k in range(KH):
                    ps = ps_mm.tile([P, P], BF16, tag="mm", name=f"xtp{b}_{k}")
                    nc.tensor.transpose(ps[:], xbf[:, P * k:P * (k + 1)],
                                        ident[:])
                    nc.vector.tensor_copy(xT[:, k, P * b:P * (b + 1)], ps[:])

            # ---- W_gu pair 0 leads the sync ring (f32 + vector cast) ----
            wgu_stage = {}
            wgub_tiles = {}

            def w_src(f0):
                return wgu_d[:, f0:f0 + P].rearrange("(ko ki) f -> ki ko f", ki=P)

            def emit_w_dma(i):
                wg = wgu_pool.tile([P, KH, P], F32, tag="wgu", name=f"wg{i}")
                wu = wgu_pool.tile([P, KH, P], F32, tag="wgu", name=f"wu{i}")
                nc.sync.dma_start(wg[:], w_src(P * i))
                nc.sync.dma_start(wu[:], w_src(P * i + I))
                wgu_stage[i] = (wg, wu)

            def emit_w_cast(i):
                wg, wu = wgu_stage.pop(i)
                wgb = wgub_pool.tile([P, KH, P], BF16, tag="wgub", name=f"wgb{i}")
                wub = wgub_pool.tile([P, KH, P], BF16, tag="wgub", name=f"wub{i}")
                nc.vector.tensor_copy(wgb[:], wg[:])
                nc.vector.tensor_copy(wub[:], wu[:])
                wgub_tiles[i] = (wgb, wub)

            ident = small_pool.tile([P, P], BF16, tag="ident")
            make_identity(nc, ident[:])

            emit_w_dma(0)
            x_load(0); x_load(1); x_load(2); x_load(3)
            x_load(4); x_load(5); x_load(6); x_load(7)
            x_store(4); x_store(5); x_store(6); x_store(7)
            emit_w_dma(1)
            emit_w_dma(2)
            # W0 cast on scalar: vector is busy with the x casts the PE
            # transposes are waiting on.
            wg0, wu0 = wgu_stage.pop(0)
            wgb0 = wgub_pool.tile([P, KH, P], BF16, tag="wgub", name="wgb0")
            wub0 = wgub_pool.tile([P, KH, P], BF16, tag="wgub", name="wub0")
            nc.scalar.activation(wgb0[:], wg0[:],
                                 mybir.ActivationFunctionType.Copy)
            nc.scalar.activation(wub0[:], wu0[:],
                                 mybir.ActivationFunctionType.Copy)
            wgub_tiles[0] = (wgb0, wub0)
            # t=0 half of x transposed on the PE
            x_transpose_pe(0, ident)
            x_transpose_pe(1, ident)
            x_transpose_pe(2, ident)
            x_transpose_pe(3, ident)

            # ---- r1T = SCALING * (A_gu @ x^T): [8 R, 1024 T] bf16 ----
            r1T = small_pool.tile([R, T], BF16, tag="r1T")

            def emit_r1(t):
                rps = ps_mm.tile([R, NFREE], F32, tag="mm", name=f"r1ps{t}")
                for k in range(KH):
                    nc.tensor.matmul(rps[:], aguLb[:, R * k:R * (k + 1)],
                                     xT[:, k, NFREE * t:NFREE * (t + 1)],
                                     start=(k == 0), stop=(k == KH - 1))
                nc.vector.tensor_scalar_mul(
                    r1T[:, NFREE * t:NFREE * (t + 1)], rps[:], SCALING)

            # ---- phase B: gate_up^T + SwiGLU -> hT[k]; r2 folded in ----
            hT = [hT_pool.tile([P, T], BF16, tag=f"hT{k}", name=f"hT{k}") for k in range(KI)]
            wdnb = [wdnb_pool.tile([P, H], BF16, tag=f"wdnb{k}", name=f"wdnb{k}")
                    for k in range(KI)]
            r2T = small_pool.tile([R, T], BF16, tag="r2T")
            rps2 = [ps_r2.tile([R, NFREE], F32, tag="r2", name=f"r2ps{t}")
                    for t in range(NT)]

            def iter_block(i, t):
                fg, fu = P * i, P * i + I
                wgb, wub = wgub_tiles[i]
                ts = slice(NFREE * t, NFREE * (t + 1))
                psg = ps_mm.tile([P, NFREE], F32, tag="mm", name=f"psg{i}_{t}")
                psu = ps_mm.tile([P, NFREE], F32, tag="mm", name=f"psu{i}_{t}")
                for k in range(KH):
                    nc.tensor.matmul(psg[:], wgb[:, k, :],
                                     xT[:, k, ts], start=(k == 0), stop=False)
                nc.tensor.matmul(psg[:], bguTb[:, fg:fg + P], r1T[:, ts],
                                 start=False, stop=True)
                for k in range(KH):
                    nc.tensor.matmul(psu[:], wub[:, k, :],
                                     xT[:, k, ts], start=(k == 0), stop=False)
                nc.tensor.matmul(psu[:], bguTb[:, fu:fu + P], r1T[:, ts],
                                 start=False, stop=True)
                sg = silu_pool.tile([P, NFREE], F32, tag="silu")
                nc.scalar.activation(sg[:], psg[:],
                                     mybir.ActivationFunctionType.Silu)
                nc.vector.tensor_mul(hT[i][:, ts], sg[:], psu[:])

            def iter_tail(i):
                # prefetch W pair i+4 (after the blocks whose wgu-staging
                # slots it reuses have been emitted), pace W_dn casting DMAs
                # on the gpsimd ring, and fold iter i's r2 accumulation in.
                if i + 4 <= 15:
                    emit_w_dma(i + 4)
                if 2 <= i < 10:
                    for k in (2 * (i - 2), 2 * (i - 2) + 1):
                        nc.gpsimd.dma_start(wdnb[k][:], wdn_d[P * k:P * (k + 1), :])
                for t in range(NT):
                    nc.tensor.matmul(rps2[t][:], adnLb[:, R * i:R * (i + 1)],
                                     hT[i][:, NFREE * t:NFREE * (t + 1)],
                                     start=(i == 0), stop=(i == KI - 1))

            # t=0 sweep over iters 0-3 first (PE-transposed x half), giving
            # the XBAR time to deliver the t=1 half.
            emit_r1(0)
            iter_block(0, 0)
            emit_w_dma(3)
            emit_w_cast(1)
            iter_block(1, 0)
            emit_w_cast(2)
            iter_block(2, 0)
            emit_w_cast(3)
            iter_block(3, 0)
            emit_r1(1)
            iter_block(0, 1); iter_tail(0)
            iter_block(1, 1); iter_tail(1)
            iter_block(2, 1); iter_tail(2)
            iter_block(3, 1); iter_tail(3)
            for i in range(4, NFP):
                emit_w_cast(i)
                iter_block(i, 0)
                iter_block(i, 1)
                iter_tail(i)

            for t in range(NT):
                nc.vector.tensor_scalar_mul(
                    r2T[:, NFREE * t:NFREE * (t + 1)], rps2[t][:], SCALING)

            # ---- phase D: out[T, H] = hidden @ W_dn + lora ----
            # 8 groups of one T-column each: stores spread evenly, short tail.
            nout = 0
            for j in range(8):
                po = [ps_mm.tile([P, NFREE], F32, tag="mm", name=f"po{j}_{h}")
                      for h in range(NH)]
                for k in range(KI):
                    for h in range(NH):
                        hs = slice(NFREE * h, NFREE * (h + 1))
                        nc.tensor.matmul(po[h][:],
                                         hT[k][:, P * j:P * (j + 1)],
                                         wdnb[k][:, hs],
                                         start=(k == 0), stop=False)
                for h in range(NH):
                    hs = slice(NFREE * h, NFREE * (h + 1))
                    nc.tensor.matmul(po[h][:], r2T[:, P * j:P * (j + 1)],
                                     bdnTb[:, hs], start=False, stop=True)
                    ot = out_pool.tile([P, NFREE], F32, tag="outs")
                    if nout % 2 == 0:
                        nc.scalar.activation(ot[:], po[h][:],
                                             mybir.ActivationFunctionType.Copy)
                    else:
                        nc.vector.tensor_copy(ot[:], po[h][:])
                    nout += 1
                    nc.sync.dma_start(out_d[P * j:P * (j + 1), hs], ot[:])

    nc.finalize()
    return nc


_NC_CACHE = None


def _get_nc():
    global _NC_CACHE
    if _NC_CACHE is None:
        _NC_CACHE = build_kernel()
    return _NC_CACHE


def _run(hidden_states, gate_up_proj, down_proj,
         lora_A_gu, lora_B_gu, lora_A_dn, lora_B_dn, **spmd_kwargs):
    f32 = np.float32
    hidden_states = np.ascontiguousarray(hidden_states, dtype=f32)
    gate_up_proj = np.ascontiguousarray(gate_up_proj, dtype=f32)
    down_proj = np.ascontiguousarray(down_proj, dtype=f32)
    lora_A_gu = np.asarray(lora_A_gu, dtype=f32)
    lora_B_gu = np.asarray(lora_B_gu, dtype=f32)
    lora_A_dn = np.asarray(lora_A_dn, dtype=f32)
    lora_B_dn = np.asarray(lora_B_dn, dtype=f32)

    # host-side layout marshalling of the tiny (<=128KB) lora mats:
    # smA[p, R*k+r] = A[r, 128k+p] (gu cols then dn cols); smB = [B_gu.T | B_dn.T]
    a_guL = lora_A_gu.reshape(R, KH, P).transpose(2, 1, 0).reshape(P, R * KH)
    a_dnL = lora_A_dn.reshape(R, KI, P).transpose(2, 1, 0).reshape(P, R * KI)
    smA = np.ascontiguousarray(np.concatenate([a_guL, a_dnL], axis=1))
    smB = np.ascontiguousarray(np.concatenate([lora_B_gu.T, lora_B_dn.T], axis=1))

    nc = _get_nc()
    in_maps = []
    for e in range(E):
        in_maps.append({
            "x": hidden_states[T * e:T * (e + 1), :],
            "w_gu": gate_up_proj[e],
            "w_dn": down_proj[e],
            "smA": smA,
            "smB": smB,
        })
    res = run_bass_kernel_spmd(nc, in_maps, core_ids=list(range(E)),
                               **spmd_kwargs)
    out = np.concatenate([res.results[e]["out"] for e in range(E)], axis=0)
    return out, res


def kernel(hidden_states, gate_up_proj, down_proj,
           lora_A_gu, lora_B_gu, lora_A_dn, lora_B_dn):
    out, _ = _run(hidden_states, gate_up_proj, down_proj,
                  lora_A_gu, lora_B_gu, lora_A_dn, lora_B_dn)
    return out
